# revision 1
# baseline (speedup 1.0000x reference)
"""EngramGating Trainium2 Bass kernel.

Reference computation (per token t, head h, DIM=32, HC_MULT=4):
    key[t,h,:]  = emb[t,:] @ Wk[h].T + bk[h]                  # [4,32]
    nk = key * rsqrt(mean_k(key^2)+eps) * g1
    nq = hid  * rsqrt(mean_k(hid^2)+eps) * g2
    gate0[t,h] = sum_k nk*nq / sqrt(32)
    ga = sign(gate0)*sqrt(max(|gate0|,1e-6));  gate = sigmoid(ga)
    out[t,h,:] = gate[t,h] * (emb[t,:] @ Wv.T + bv)

Sharding: pure data parallel over 8 cores, contiguous token ranges.

Per-core layout: tokens-on-partitions. Each block covers 2304 tokens
(18 tokens per partition = 6 chunks x 3 tokens). emb chunks [128,96]
are PE-transposed into persistent embT tiles whose rows 96:128 are
all-ones; one K=128 matmul per chunk against a block-diagonal
[Wk|Wv] + bias-row constant produces key|value (with biases) in PSUM.
(fp32 matmuls cannot accumulate across row-tiles on this stack, hence
block-diagonal instead of K=32 row tiling.) ACT evacuates PSUM->SBUF
and does most squares; DVE does the three segmented reductions
(sum_k key^2, hid^2, key*hid) plus part of the squares/finals; GPSIMD
does the key*hid products and most of the final gate*value. The
scalar tail (sqrt/sign/sigmoid) is batched per superblock to amortize
ACT table-set loads, with a 1-block last superblock and an even
DVE/GPSIMD final split there to shorten the end-of-kernel chain.
"""

import math
import numpy as np
from contextlib import ExitStack

import concourse.bass as bass
import concourse.bacc as bacc
import concourse.mybir as mybir
import concourse.tile as tile
from concourse.bass_utils import run_bass_kernel_spmd

F32 = mybir.dt.float32
AF = mybir.ActivationFunctionType
ALU = mybir.AluOpType
AX = mybir.AxisListType

# problem dims
B, S, DIM, H = 16, 16384, 32, 4
TOK = B * S                  # 262144
NCORES = 8
TPC = TOK // NCORES          # 32768 tokens per core
HK = H * DIM                 # 128

# block geometry
TPP = 18                     # tokens per partition per block (6 chunks x 3)
BLK = 128 * TPP              # 2304 tokens per block
NCHUNK = 6                   # chunks per block (3 tokens each per partition)
NPAIR = 3                    # chunk pairs
SB_SIZES = [5, 5, 3, 2]      # blocks per superblock (scalar-tail batch);
                             # small last superblock hides the end tail
SB_BLKS = 5                  # max superblock size (staging tile size)
EPS = float(np.finfo(np.float32).eps)

# 14 full blocks + 1 short (tpp=6) block covering the [TPC-768, TPC)
# remainder (256-token overlap). The short block is scheduled mid-stream
# (its own 1-block superblock) so the kernel ends on a well-pipelined
# full superblock.
T0S = [i * BLK for i in range(TPC // BLK)] + [TPC - 128 * 6]
TPPS = [TPP] * (TPC // BLK) + [6]
NBLK = len(T0S)              # 15
assert sum(SB_SIZES) == NBLK


def _build_nc(apply_g12: bool, reps: int = 1):
    nc = bacc.Bacc(None, target_bir_lowering=False, debug=False)

    emb_d = nc.dram_tensor("emb", [TPC * DIM], F32, kind="ExternalInput")
    hid_d = nc.dram_tensor("hid", [TPC * HK], F32, kind="ExternalInput")
    wkv_d = nc.dram_tensor("wkv", [128, 480], F32, kind="ExternalInput")
    ident_d = nc.dram_tensor("ident", [128, 128], F32, kind="ExternalInput")
    g12_d = None
    if apply_g12:
        g12_d = nc.dram_tensor("g12", [128, HK], F32, kind="ExternalInput")
    out_d = nc.dram_tensor("out", [TPC * HK], F32, kind="ExternalOutput")

    with tile.TileContext(nc) as tc, ExitStack() as ctx:
        const_p = ctx.enter_context(tc.tile_pool(name="const", bufs=1))
        emb_p = ctx.enter_context(tc.tile_pool(name="embp", bufs=2))
        hid_p = ctx.enter_context(tc.tile_pool(name="hidp", bufs=2))
        tp_p = ctx.enter_context(
            tc.tile_pool(name="tpp", bufs=2, space=bass.MemorySpace.PSUM))
        kvp_p = ctx.enter_context(
            tc.tile_pool(name="kvpp", bufs=2, space=bass.MemorySpace.PSUM))
        kvsb_p = ctx.enter_context(tc.tile_pool(name="kvsbp", bufs=2))
        sq_p = ctx.enter_context(tc.tile_pool(name="sqp", bufs=2))
        prod_p = ctx.enter_context(tc.tile_pool(name="prodp", bufs=2))
        stage_p = ctx.enter_context(tc.tile_pool(name="stagep", bufs=2))
        tail_p = ctx.enter_context(tc.tile_pool(name="tailp", bufs=1))
        out_p = ctx.enter_context(tc.tile_pool(name="outp", bufs=2))

        wkv_sb = const_p.tile([128, 480], F32)
        ident_sb = const_p.tile([128, 128], F32)
        eps_k = const_p.tile([128, 1], F32)
        eps_q = const_p.tile([128, 1], F32)
        nc.gpsimd.memset(eps_k[:], 32.0 * EPS)
        nc.gpsimd.memset(eps_q[:], EPS)
        nc.sync.dma_start(wkv_sb[:], wkv_d[:])
        nc.sync.dma_start(ident_sb[:], ident_d[:])
        if apply_g12:
            g12_sb = const_p.tile([128, HK], F32)
            nc.sync.dma_start(g12_sb[:], g12_d[:])

        # persistent embT tiles: rows 96:128 stay all-ones (bias rows for
        # the K=128 block-diagonal matmul); rows 0:96 rewritten per pair.
        embT_tiles = []
        for i in range(3):
            t = const_p.tile([128, 2, 128], F32, name=f"embT{i}")
            nc.gpsimd.memset(t[96:128, :, :], 1.0)
            embT_tiles.append(t)

        starts = []
        acc = 0
        for sz in SB_SIZES:
            starts.append(acc)
            acc += sz
        sbs = [(s, sz) for _ in range(reps) for s, sz in zip(starts, SB_SIZES)]

        def emit_block(b, bb, msk_st, msq_st, dot_st, val_st):
            if True:
                t0 = T0S[b]
                tpp = TPPS[b]
                blk = 128 * tpp
                npair = tpp // 6

                emb_sb = emb_p.tile([128, tpp * DIM], F32, name="emb_sb")
                nc.sync.dma_start(
                    emb_sb[:],
                    emb_d[t0 * DIM:(t0 + blk) * DIM].rearrange(
                        "(p f) -> p f", p=128))
                hid_sb = hid_p.tile([128, tpp * HK], F32, name="hid_sb")
                nc.sync.dma_start(
                    hid_sb[:],
                    hid_d[t0 * HK:(t0 + blk) * HK].rearrange(
                        "(p f) -> p f", p=128))

                kv_sb = kvsb_p.tile([128, tpp, HK], F32, name="kv_sb")

                # phase 1: all transposes (PE) + embT copies (ACT) so the
                # in-order ACT stream isn't blocked behind evacs waiting on
                # matmuls of earlier pairs.
                tps = []
                for g in range(npair):
                    tp = tp_p.tile([96, 2, 128], F32, name="tp", bufs=3)
                    for c2 in range(2):
                        cc = 2 * g + c2
                        nc.tensor.matmul(
                            tp[:, c2, :],
                            emb_sb[:, 96 * cc:96 * (cc + 1)],
                            ident_sb[:],
                            is_transpose=True,
                            start=(c2 == 0), stop=(c2 == 1))
                    tps.append(tp)
                for g in range(npair):
                    nc.scalar.copy(embT_tiles[g][0:96, :, :], tps[g][:])

                # phase 2: matmuls (PE) interleaved with evacs (ACT)
                for g in range(npair):
                    kvp = kvp_p.tile([128, 2, 512], F32, name="kvp")
                    for c2 in range(2):
                        # single K=128 matmul: rows 0:96 = 3 transposed
                        # token-groups against block-diagonal W, rows
                        # 96:128 = ones against the bias row.
                        nc.tensor.matmul(
                            kvp[:, c2, 0:480],
                            embT_tiles[g][:, c2, :],
                            wkv_sb[:, 0:480],
                            start=True, stop=True)
                    # evacuate PSUM -> SBUF (ACT): key and val parts
                    kvp4 = kvp[:, :, 0:480].rearrange(
                        "p c (j m) -> p c j m", m=160)
                    nc.scalar.copy(
                        kv_sb[:, 6 * g:6 * (g + 1), :].rearrange(
                            "p (c j) m -> p c j m", c=2),
                        kvp4[:, :, :, 0:HK])
                    nc.scalar.copy(
                        val_st[:, bb, 6 * g:6 * (g + 1), :].rearrange(
                            "p (c j) m -> p c j m", c=2),
                        kvp4[:, :, :, HK:160])

                key4 = kv_sb[:].rearrange("p s (h k) -> p s h k", h=H)
                hid4 = hid_sb.rearrange("p (s h k) -> p s h k", s=tpp, h=H)

                sqk = sq_p.tile([128, tpp, H, DIM], F32, name="sqk")
                nc.scalar.activation(sqk[:], key4, AF.Square)
                sqq = sq_p.tile([128, tpp, H, DIM], F32, name="sqq")
                # split hid^2 between ACT (busiest engine) and DVE
                QSPL = min(8, tpp)
                nc.vector.tensor_tensor(
                    sqq[:, 0:QSPL], hid4[:, 0:QSPL], hid4[:, 0:QSPL],
                    op=ALU.mult)
                if QSPL < tpp:
                    nc.scalar.activation(sqq[:, QSPL:tpp], hid4[:, QSPL:tpp],
                                         AF.Square)

                if apply_g12:
                    prod_in1 = prod_p.tile([128, tpp, H, DIM], F32, name="hidg")
                    nc.vector.tensor_tensor(
                        prod_in1[:], hid4,
                        g12_sb[:].rearrange("p (o h k) -> p o h k", o=1, h=H)
                        .broadcast_to([128, tpp, H, DIM]),
                        op=ALU.mult)
                    prod_in1 = prod_in1[:]
                else:
                    prod_in1 = hid4

                prod = prod_p.tile([128, tpp, H, DIM], F32, name="prod")
                nc.gpsimd.tensor_tensor(prod[:], key4, prod_in1, op=ALU.mult)

                # red_q first: its input (own sqq) is ready earliest
                nc.vector.reduce_sum(msq_st[:, bb, 0:tpp, :], sqq[:], axis=AX.X)
                nc.vector.reduce_sum(msk_st[:, bb, 0:tpp, :], sqk[:], axis=AX.X)
                nc.vector.reduce_sum(dot_st[:, bb, 0:tpp, :], prod[:], axis=AX.X)
                if tpp < TPP:
                    # pad unused staging slots so the superblock tail can
                    # process the full range (results are discarded)
                    nc.gpsimd.memset(msk_st[:, bb, tpp:TPP, :], 1.0)
                    nc.gpsimd.memset(msq_st[:, bb, tpp:TPP, :], 1.0)
                    nc.gpsimd.memset(dot_st[:, bb, tpp:TPP, :], 1.0)

        def emit_tail_finals(sb0, sb_sz, msk_st, msq_st, dot_st, val_st,
                             is_last):
            # ---- superblock scalar tail ----
            # |g0| = |dot|/(sk*sq2);  gate = 0.5 + sign(dot)*(sig(r)-0.5)
            # ordered to minimize ACT<->DVE alternations (in-order engines)
            ft_tpp = TPP
            FT = sb_sz * ft_tpp * H
            msk_f = msk_st[:, 0:sb_sz, 0:ft_tpp].rearrange(
                "p a b c -> p (a b c)")
            msq_f = msq_st[:, 0:sb_sz, 0:ft_tpp].rearrange(
                "p a b c -> p (a b c)")
            dot_f = dot_st[:, 0:sb_sz, 0:ft_tpp].rearrange(
                "p a b c -> p (a b c)")
            sk = tail_p.tile([128, FT], F32, name="sk", tag="sk")
            nc.scalar.activation(sk[:], msk_f, AF.Sqrt, bias=eps_k[:])
            sq2 = tail_p.tile([128, FT], F32, name="sq2", tag="sq2")
            nc.scalar.activation(sq2[:], msq_f, AF.Sqrt,
                                 bias=eps_q[:], scale=1.0 / 32.0)
            sg = tail_p.tile([128, FT], F32, name="sg", tag="sg")
            nc.scalar.activation(sg[:], dot_f, AF.Sign)
            aa = tail_p.tile([128, FT], F32, name="aa", tag="aa")
            nc.scalar.activation(aa[:], dot_f, AF.Abs)
            den = tail_p.tile([128, FT], F32, name="den", tag="den")
            nc.vector.tensor_tensor(den[:], sk[:], sq2[:], op=ALU.mult)
            rden = tail_p.tile([128, FT], F32, name="rden", tag="rden")
            nc.vector.reciprocal(rden[:], den[:])
            mm_t = tail_p.tile([128, FT], F32, name="mm_t", tag="mm_t")
            nc.vector.tensor_tensor(mm_t[:], aa[:], rden[:], op=ALU.mult)
            m = tail_p.tile([128, FT], F32, name="m", tag="m")
            nc.vector.tensor_scalar(m[:], mm_t[:], 1e-6, None, op0=ALU.max)
            r = tail_p.tile([128, FT], F32, name="r", tag="r")
            nc.scalar.activation(r[:], m[:], AF.Sqrt)
            sr = tail_p.tile([128, FT], F32, name="sr", tag="sr")
            nc.scalar.activation(sr[:], r[:], AF.Sigmoid)
            gate = tail_p.tile([128, SB_BLKS, TPP, H], F32, name="gate")
            g5 = gate[:, 0:sb_sz, 0:ft_tpp].rearrange("p a b c -> p (a b c)")
            nc.vector.scalar_tensor_tensor(
                g5, sr[:], -0.5, sg[:], op0=ALU.add, op1=ALU.mult)
            nc.vector.tensor_scalar(g5, g5, 0.5, None, op0=ALU.add)

            # ---- final gating + store ----
            for bb in range(sb_sz):
                b = sb0 + bb
                t0 = T0S[b]
                tpp = TPPS[b]
                blk = 128 * tpp
                out_sb = out_p.tile([128, tpp, H, DIM], F32, name="out_sb")
                gate_b = gate[:, bb, 0:tpp, :].unsqueeze(3)
                val_b = val_st[:, bb, 0:tpp, :].unsqueeze(2)
                # split final elementwise mul between DVE and GPSIMD; in the
                # last superblock DVE is idle, so split evenly to shorten the
                # end-of-kernel chain
                SPL = min(9 if is_last else 7, tpp)
                nc.vector.tensor_tensor(
                    out_sb[:, 0:SPL, :, :],
                    gate_b[:, 0:SPL, :, :].broadcast_to([128, SPL, H, DIM]),
                    val_b[:, 0:SPL, :, :].broadcast_to([128, SPL, H, DIM]),
                    op=ALU.mult)
                if SPL < tpp:
                    nc.gpsimd.tensor_tensor(
                        out_sb[:, SPL:tpp, :, :],
                        gate_b[:, SPL:tpp, :, :].broadcast_to(
                            [128, tpp - SPL, H, DIM]),
                        val_b[:, SPL:tpp, :, :].broadcast_to(
                            [128, tpp - SPL, H, DIM]),
                        op=ALU.mult)
                nc.sync.dma_start(
                    out_d[t0 * HK:(t0 + blk) * HK].rearrange(
                        "(p f) -> p f", p=128),
                    out_sb[:].rearrange("p a b c -> p (a b c)"))

        for sb_i, (sb0, sb_sz) in enumerate(sbs):
            # superblock staging
            msk_st = stage_p.tile([128, SB_BLKS, TPP, H], F32, name="msk_st")
            msq_st = stage_p.tile([128, SB_BLKS, TPP, H], F32, name="msq_st")
            dot_st = stage_p.tile([128, SB_BLKS, TPP, H], F32, name="dot_st")
            val_st = stage_p.tile([128, SB_BLKS, TPP, DIM], F32,
                                  name="val_st")
            for bb in range(sb_sz):
                emit_block(sb0 + bb, bb, msk_st, msq_st, dot_st, val_st)
            emit_tail_finals(sb0, sb_sz, msk_st, msq_st, dot_st, val_st,
                             sb_i == len(sbs) - 1)

    nc.compile()
    return nc


def _prep_consts(Wv, bv, Wk, bk):
    # Wkv_cat[d, h*32+k] = Wk[h,k,d];  Wkv_cat[d, 128+v] = Wv[v,d]
    wkv_cat = np.zeros((DIM, 160), dtype=np.float32)
    wkv_cat[:, 0:HK] = np.transpose(Wk, (2, 0, 1)).reshape(DIM, HK)
    wkv_cat[:, HK:160] = Wv.T
    bias_cat = np.concatenate(
        [bk.reshape(HK).astype(np.float32), bv.astype(np.float32)])
    wkv = np.zeros((128, 480), dtype=np.float32)
    for j in range(3):
        wkv[32 * j:32 * (j + 1), 160 * j:160 * (j + 1)] = wkv_cat
    wkv[96, :] = np.tile(bias_cat, 3)
    ident = np.eye(128, dtype=np.float32)
    return wkv, ident


_CACHE = {}


def kernel_with_results(embeddings, hidden_states, Wv, bv, Wk, bk, g1, g2,
                        **run_kwargs):
    embeddings = np.ascontiguousarray(np.asarray(embeddings, dtype=np.float32))
    hidden_states = np.ascontiguousarray(
        np.asarray(hidden_states, dtype=np.float32))
    Wv = np.asarray(Wv, dtype=np.float32)
    bv = np.asarray(bv, dtype=np.float32)
    Wk = np.asarray(Wk, dtype=np.float32)
    bk = np.asarray(bk, dtype=np.float32)
    g12 = (np.asarray(g1, np.float32) * np.asarray(g2, np.float32))
    apply_g12 = not np.all(g12 == 1.0)

    if apply_g12 not in _CACHE:
        _CACHE[apply_g12] = _build_nc(apply_g12)
    nc = _CACHE[apply_g12]

    wkv, ident = _prep_consts(Wv, bv, Wk, bk)

    emb_flat = embeddings.reshape(TOK, DIM)
    hid_flat = hidden_states.reshape(TOK, HK)

    in_maps = []
    for c in range(NCORES):
        m = {
            "emb": np.ascontiguousarray(
                emb_flat[c * TPC:(c + 1) * TPC]).reshape(-1),
            "hid": np.ascontiguousarray(
                hid_flat[c * TPC:(c + 1) * TPC]).reshape(-1),
            "wkv": wkv,
            "ident": ident,
        }
        if apply_g12:
            m["g12"] = np.tile(
                g12.reshape(1, HK), (128, 1)).astype(np.float32)
        in_maps.append(m)

    res = run_bass_kernel_spmd(nc, in_maps, core_ids=list(range(NCORES)),
                               **run_kwargs)
    out = np.concatenate(
        [res.results[c]["out"].reshape(TPC, HK) for c in range(NCORES)],
        axis=0)
    return out.reshape(B, S, H, DIM), res


def kernel(embeddings, hidden_states, Wv, bv, Wk, bk, g1, g2):
    out, _ = kernel_with_results(
        embeddings, hidden_states, Wv, bv, Wk, bk, g1, g2)
    return out



# revision 6
# speedup vs baseline: 1.0920x; 1.0920x over previous
"""EngramGating Trainium2 Bass kernel.

Reference computation (per token t, head h, DIM=32, HC_MULT=4):
    key[t,h,:]  = emb[t,:] @ Wk[h].T + bk[h]                  # [4,32]
    nk = key * rsqrt(mean_k(key^2)+eps) * g1
    nq = hid  * rsqrt(mean_k(hid^2)+eps) * g2
    gate0[t,h] = sum_k nk*nq / sqrt(32)
    ga = sign(gate0)*sqrt(max(|gate0|,1e-6));  gate = sigmoid(ga)
    out[t,h,:] = gate[t,h] * (emb[t,:] @ Wv.T + bv)

Sharding: pure data parallel over 8 cores, contiguous token ranges.

Per-core layout: tokens-on-partitions. Each block covers 2304 tokens
(18 tokens per partition = 6 chunks x 3 tokens). emb chunks [128,96]
are PE-transposed into persistent embT tiles whose rows 96:128 are
all-ones; one K=128 matmul per chunk against a block-diagonal
[Wk|Wv] + bias-row constant produces key|value (with biases) in PSUM.
(fp32 matmuls cannot accumulate across row-tiles on this stack, hence
block-diagonal instead of K=32 row tiling.) ACT evacuates PSUM->SBUF
and does most squares; DVE does the three segmented reductions
(sum_k key^2, hid^2, key*hid) plus part of the squares/finals; GPSIMD
does the key*hid products and most of the final gate*value. The
scalar tail (sqrt/sign/sigmoid) is batched per superblock to amortize
ACT table-set loads, with a 1-block last superblock and an even
DVE/GPSIMD final split there to shorten the end-of-kernel chain.
"""

import math
import numpy as np
from contextlib import ExitStack

import concourse.bass as bass
import concourse.bacc as bacc
import concourse.mybir as mybir
import concourse.tile as tile
from concourse.bass_utils import run_bass_kernel_spmd

F32 = mybir.dt.float32
AF = mybir.ActivationFunctionType
ALU = mybir.AluOpType
AX = mybir.AxisListType

# problem dims
B, S, DIM, H = 16, 16384, 32, 4
TOK = B * S                  # 262144
NCORES = 8
TPC = TOK // NCORES          # 32768 tokens per core
HK = H * DIM                 # 128

# block geometry
TPP = 18                     # tokens per partition per block (6 chunks x 3)
BLK = 128 * TPP              # 2304 tokens per block
NCHUNK = 6                   # chunks per block (3 tokens each per partition)
NPAIR = 3                    # chunk pairs
SB_SIZES = [5, 5, 3, 2]      # blocks per superblock (scalar-tail batch);
                             # small last superblock hides the end tail
SB_BLKS = 5                  # max superblock size (staging tile size)
EPS = float(np.finfo(np.float32).eps)

# 14 full blocks + 1 short (tpp=6) block covering the [TPC-768, TPC)
# remainder (256-token overlap). The short block is scheduled mid-stream
# (its own 1-block superblock) so the kernel ends on a well-pipelined
# full superblock.
T0S = [i * BLK for i in range(TPC // BLK)] + [TPC - 128 * 6]
TPPS = [TPP] * (TPC // BLK) + [6]
NBLK = len(T0S)              # 15
assert sum(SB_SIZES) == NBLK


def _build_nc(apply_g12: bool, reps: int = 1):
    nc = bacc.Bacc(None, target_bir_lowering=False, debug=False)

    emb_d = nc.dram_tensor("emb", [TPC * DIM], F32, kind="ExternalInput")
    hid_d = nc.dram_tensor("hid", [TPC * HK], F32, kind="ExternalInput")
    wkv_d = nc.dram_tensor("wkv", [128, 480], F32, kind="ExternalInput")
    ident_d = nc.dram_tensor("ident", [128, 128], F32, kind="ExternalInput")
    g12_d = None
    if apply_g12:
        g12_d = nc.dram_tensor("g12", [128, HK], F32, kind="ExternalInput")
    out_d = nc.dram_tensor("out", [TPC * HK], F32, kind="ExternalOutput")

    with tile.TileContext(nc) as tc, ExitStack() as ctx:
        const_p = ctx.enter_context(tc.tile_pool(name="const", bufs=1))
        emb_p = ctx.enter_context(tc.tile_pool(name="embp", bufs=2))
        hid_p = ctx.enter_context(tc.tile_pool(name="hidp", bufs=2))
        tp_p = ctx.enter_context(
            tc.tile_pool(name="tpp", bufs=2, space=bass.MemorySpace.PSUM))
        kvp_p = ctx.enter_context(
            tc.tile_pool(name="kvpp", bufs=2, space=bass.MemorySpace.PSUM))
        kvsb_p = ctx.enter_context(tc.tile_pool(name="kvsbp", bufs=2))
        sq_p = ctx.enter_context(tc.tile_pool(name="sqp", bufs=2))
        prod_p = ctx.enter_context(tc.tile_pool(name="prodp", bufs=2))
        stage_p = ctx.enter_context(tc.tile_pool(name="stagep", bufs=2))
        tail_p = ctx.enter_context(tc.tile_pool(name="tailp", bufs=1))
        out_p = ctx.enter_context(tc.tile_pool(name="outp", bufs=2))

        wkv_sb = const_p.tile([128, 480], F32)
        ident_sb = const_p.tile([128, 128], F32)
        eps_k = const_p.tile([128, 1], F32)
        eps_q = const_p.tile([128, 1], F32)
        nc.gpsimd.memset(eps_k[:], 32.0 * EPS)
        nc.gpsimd.memset(eps_q[:], EPS)
        nc.sync.dma_start(wkv_sb[:], wkv_d[:])
        nc.sync.dma_start(ident_sb[:], ident_d[:])
        if apply_g12:
            g12_sb = const_p.tile([128, HK], F32)
            nc.sync.dma_start(g12_sb[:], g12_d[:])

        # persistent embT tiles: rows 96:128 stay all-ones (bias rows for
        # the K=128 block-diagonal matmul); rows 0:96 rewritten per pair.
        embT_tiles = []
        for i in range(3):
            t = const_p.tile([128, 2, 128], F32, name=f"embT{i}")
            nc.gpsimd.memset(t[96:128, :, :], 1.0)
            embT_tiles.append(t)

        starts = []
        acc = 0
        for sz in SB_SIZES:
            starts.append(acc)
            acc += sz
        sbs = [(s, sz) for _ in range(reps) for s, sz in zip(starts, SB_SIZES)]

        def emit_block(b, bb, msk_st, msq_st, dot_st, val_st):
            if True:
                t0 = T0S[b]
                tpp = TPPS[b]
                blk = 128 * tpp
                npair = tpp // 6

                emb_sb = emb_p.tile([128, tpp * DIM], F32, name="emb_sb")
                nc.sync.dma_start(
                    emb_sb[:],
                    emb_d[t0 * DIM:(t0 + blk) * DIM].rearrange(
                        "(p f) -> p f", p=128))
                hid_sb = hid_p.tile([128, tpp * HK], F32, name="hid_sb")
                nc.sync.dma_start(
                    hid_sb[:],
                    hid_d[t0 * HK:(t0 + blk) * HK].rearrange(
                        "(p f) -> p f", p=128))

                kv_sb = kvsb_p.tile([128, tpp, HK], F32, name="kv_sb")

                # phase 1: all transposes (PE) + embT copies (ACT) so the
                # in-order ACT stream isn't blocked behind evacs waiting on
                # matmuls of earlier pairs.
                tps = []
                for g in range(npair):
                    tp = tp_p.tile([96, 2, 128], F32, name="tp", bufs=3)
                    for c2 in range(2):
                        cc = 2 * g + c2
                        nc.tensor.matmul(
                            tp[:, c2, :],
                            emb_sb[:, 96 * cc:96 * (cc + 1)],
                            ident_sb[:],
                            is_transpose=True,
                            start=(c2 == 0), stop=(c2 == 1))
                    tps.append(tp)
                for g in range(npair):
                    nc.scalar.copy(embT_tiles[g][0:96, :, :], tps[g][:])

                # phase 2: matmuls (PE) interleaved with evacs (ACT)
                for g in range(npair):
                    kvp = kvp_p.tile([128, 2, 512], F32, name="kvp")
                    for c2 in range(2):
                        # single K=128 matmul: rows 0:96 = 3 transposed
                        # token-groups against block-diagonal W, rows
                        # 96:128 = ones against the bias row.
                        nc.tensor.matmul(
                            kvp[:, c2, 0:480],
                            embT_tiles[g][:, c2, :],
                            wkv_sb[:, 0:480],
                            start=True, stop=True)
                    # evacuate PSUM -> SBUF (ACT): key and val parts
                    kvp4 = kvp[:, :, 0:480].rearrange(
                        "p c (j m) -> p c j m", m=160)
                    nc.scalar.copy(
                        kv_sb[:, 6 * g:6 * (g + 1), :].rearrange(
                            "p (c j) m -> p c j m", c=2),
                        kvp4[:, :, :, 0:HK])
                    nc.scalar.copy(
                        val_st[:, bb, 6 * g:6 * (g + 1), :].rearrange(
                            "p (c j) m -> p c j m", c=2),
                        kvp4[:, :, :, HK:160])

                key4 = kv_sb[:].rearrange("p s (h k) -> p s h k", h=H)
                hid4 = hid_sb.rearrange("p (s h k) -> p s h k", s=tpp, h=H)

                sqk = sq_p.tile([128, tpp, H, DIM], F32, name="sqk")
                nc.scalar.activation(sqk[:], key4, AF.Square)
                sqq = sq_p.tile([128, tpp, H, DIM], F32, name="sqq")
                # split hid^2 between ACT (busiest engine) and DVE
                QSPL = min(8, tpp)
                nc.vector.tensor_tensor(
                    sqq[:, 0:QSPL], hid4[:, 0:QSPL], hid4[:, 0:QSPL],
                    op=ALU.mult)
                if QSPL < tpp:
                    nc.scalar.activation(sqq[:, QSPL:tpp], hid4[:, QSPL:tpp],
                                         AF.Square)

                if apply_g12:
                    prod_in1 = prod_p.tile([128, tpp, H, DIM], F32, name="hidg")
                    nc.vector.tensor_tensor(
                        prod_in1[:], hid4,
                        g12_sb[:].rearrange("p (o h k) -> p o h k", o=1, h=H)
                        .broadcast_to([128, tpp, H, DIM]),
                        op=ALU.mult)
                    prod_in1 = prod_in1[:]
                else:
                    prod_in1 = hid4

                prod = prod_p.tile([128, tpp, H, DIM], F32, name="prod")
                nc.gpsimd.tensor_tensor(prod[:], key4, prod_in1, op=ALU.mult)

                # red_q first: its input (own sqq) is ready earliest
                nc.vector.reduce_sum(msq_st[:, bb, 0:tpp, :], sqq[:], axis=AX.X)
                nc.vector.reduce_sum(msk_st[:, bb, 0:tpp, :], sqk[:], axis=AX.X)
                nc.vector.reduce_sum(dot_st[:, bb, 0:tpp, :], prod[:], axis=AX.X)
                if tpp < TPP:
                    # pad unused staging slots so the superblock tail can
                    # process the full range (results are discarded)
                    nc.gpsimd.memset(msk_st[:, bb, tpp:TPP, :], 1.0)
                    nc.gpsimd.memset(msq_st[:, bb, tpp:TPP, :], 1.0)
                    nc.gpsimd.memset(dot_st[:, bb, tpp:TPP, :], 1.0)

        def emit_tail_finals(sb0, sb_sz, msk_st, msq_st, dot_st, val_st,
                             is_last):
            # ---- superblock scalar tail ----
            # |g0| = |dot|/(sk*sq2);  gate = 0.5 + sign(dot)*(sig(r)-0.5)
            # ordered to minimize ACT<->DVE alternations (in-order engines)
            ft_tpp = TPP
            FT = sb_sz * ft_tpp * H
            msk_f = msk_st[:, 0:sb_sz, 0:ft_tpp].rearrange(
                "p a b c -> p (a b c)")
            msq_f = msq_st[:, 0:sb_sz, 0:ft_tpp].rearrange(
                "p a b c -> p (a b c)")
            dot_f = dot_st[:, 0:sb_sz, 0:ft_tpp].rearrange(
                "p a b c -> p (a b c)")
            sk = tail_p.tile([128, FT], F32, name="sk", tag="sk")
            nc.scalar.activation(sk[:], msk_f, AF.Sqrt, bias=eps_k[:])
            sq2 = tail_p.tile([128, FT], F32, name="sq2", tag="sq2")
            nc.scalar.activation(sq2[:], msq_f, AF.Sqrt,
                                 bias=eps_q[:], scale=1.0 / 32.0)
            sg = tail_p.tile([128, FT], F32, name="sg", tag="sg")
            nc.scalar.activation(sg[:], dot_f, AF.Sign)
            aa = tail_p.tile([128, FT], F32, name="aa", tag="aa")
            nc.scalar.activation(aa[:], dot_f, AF.Abs)
            den = tail_p.tile([128, FT], F32, name="den", tag="den")
            nc.vector.tensor_tensor(den[:], sk[:], sq2[:], op=ALU.mult)
            rden = tail_p.tile([128, FT], F32, name="rden", tag="rden")
            nc.vector.reciprocal(rden[:], den[:])
            mm_t = tail_p.tile([128, FT], F32, name="mm_t", tag="mm_t")
            nc.vector.tensor_tensor(mm_t[:], aa[:], rden[:], op=ALU.mult)
            m = tail_p.tile([128, FT], F32, name="m", tag="m")
            nc.vector.tensor_scalar(m[:], mm_t[:], 1e-6, None, op0=ALU.max)
            r = tail_p.tile([128, FT], F32, name="r", tag="r")
            nc.scalar.activation(r[:], m[:], AF.Sqrt)
            sr = tail_p.tile([128, FT], F32, name="sr", tag="sr")
            nc.scalar.activation(sr[:], r[:], AF.Sigmoid)
            gate = tail_p.tile([128, SB_BLKS, TPP, H], F32, name="gate")
            g5 = gate[:, 0:sb_sz, 0:ft_tpp].rearrange("p a b c -> p (a b c)")
            nc.vector.scalar_tensor_tensor(
                g5, sr[:], -0.5, sg[:], op0=ALU.add, op1=ALU.mult)
            nc.vector.tensor_scalar(g5, g5, 0.5, None, op0=ALU.add)

            # ---- final gating + store ----
            for bb in range(sb_sz):
                b = sb0 + bb
                t0 = T0S[b]
                tpp = TPPS[b]
                blk = 128 * tpp
                out_sb = out_p.tile([128, tpp, H, DIM], F32, name="out_sb")
                gate_b = gate[:, bb, 0:tpp, :].unsqueeze(3)
                val_b = val_st[:, bb, 0:tpp, :].unsqueeze(2)
                # split final elementwise mul between DVE and GPSIMD; in the
                # last superblock DVE is idle, so split evenly to shorten the
                # end-of-kernel chain
                SPL = min(9 if is_last else 7, tpp)
                nc.vector.tensor_tensor(
                    out_sb[:, 0:SPL, :, :],
                    gate_b[:, 0:SPL, :, :].broadcast_to([128, SPL, H, DIM]),
                    val_b[:, 0:SPL, :, :].broadcast_to([128, SPL, H, DIM]),
                    op=ALU.mult)
                if SPL < tpp:
                    nc.gpsimd.tensor_tensor(
                        out_sb[:, SPL:tpp, :, :],
                        gate_b[:, SPL:tpp, :, :].broadcast_to(
                            [128, tpp - SPL, H, DIM]),
                        val_b[:, SPL:tpp, :, :].broadcast_to(
                            [128, tpp - SPL, H, DIM]),
                        op=ALU.mult)
                nc.sync.dma_start(
                    out_d[t0 * HK:(t0 + blk) * HK].rearrange(
                        "(p f) -> p f", p=128),
                    out_sb[:].rearrange("p a b c -> p (a b c)"))

        for sb_i, (sb0, sb_sz) in enumerate(sbs):
            # superblock staging
            msk_st = stage_p.tile([128, SB_BLKS, TPP, H], F32, name="msk_st")
            msq_st = stage_p.tile([128, SB_BLKS, TPP, H], F32, name="msq_st")
            dot_st = stage_p.tile([128, SB_BLKS, TPP, H], F32, name="dot_st")
            val_st = stage_p.tile([128, SB_BLKS, TPP, DIM], F32,
                                  name="val_st")
            for bb in range(sb_sz):
                emit_block(sb0 + bb, bb, msk_st, msq_st, dot_st, val_st)
            emit_tail_finals(sb0, sb_sz, msk_st, msq_st, dot_st, val_st,
                             sb_i == len(sbs) - 1)

    nc.compile()
    return nc


def _prep_consts(Wv, bv, Wk, bk):
    # Wkv_cat[d, h*32+k] = Wk[h,k,d];  Wkv_cat[d, 128+v] = Wv[v,d]
    wkv_cat = np.zeros((DIM, 160), dtype=np.float32)
    wkv_cat[:, 0:HK] = np.transpose(Wk, (2, 0, 1)).reshape(DIM, HK)
    wkv_cat[:, HK:160] = Wv.T
    bias_cat = np.concatenate(
        [bk.reshape(HK).astype(np.float32), bv.astype(np.float32)])
    wkv = np.zeros((128, 480), dtype=np.float32)
    for j in range(3):
        wkv[32 * j:32 * (j + 1), 160 * j:160 * (j + 1)] = wkv_cat
    wkv[96, :] = np.tile(bias_cat, 3)
    ident = np.eye(128, dtype=np.float32)
    return wkv, ident


_CACHE = {}


def kernel_with_results(embeddings, hidden_states, Wv, bv, Wk, bk, g1, g2,
                        **run_kwargs):
    embeddings = np.ascontiguousarray(np.asarray(embeddings, dtype=np.float32))
    hidden_states = np.ascontiguousarray(
        np.asarray(hidden_states, dtype=np.float32))
    Wv = np.asarray(Wv, dtype=np.float32)
    bv = np.asarray(bv, dtype=np.float32)
    Wk = np.asarray(Wk, dtype=np.float32)
    bk = np.asarray(bk, dtype=np.float32)
    g12 = (np.asarray(g1, np.float32) * np.asarray(g2, np.float32))
    apply_g12 = not np.all(g12 == 1.0)

    if apply_g12 not in _CACHE:
        _CACHE[apply_g12] = _build_nc(apply_g12)
    nc = _CACHE[apply_g12]

    wkv, ident = _prep_consts(Wv, bv, Wk, bk)

    emb_flat = embeddings.reshape(TOK, DIM)
    hid_flat = hidden_states.reshape(TOK, HK)

    in_maps = []
    for c in range(NCORES):
        m = {
            "emb": np.ascontiguousarray(
                emb_flat[c * TPC:(c + 1) * TPC]).reshape(-1),
            "hid": np.ascontiguousarray(
                hid_flat[c * TPC:(c + 1) * TPC]).reshape(-1),
            "wkv": wkv,
            "ident": ident,
        }
        if apply_g12:
            m["g12"] = np.tile(
                g12.reshape(1, HK), (128, 1)).astype(np.float32)
        in_maps.append(m)

    res = run_bass_kernel_spmd(nc, in_maps, core_ids=list(range(NCORES)),
                               **run_kwargs)
    out = np.concatenate(
        [res.results[c]["out"].reshape(TPC, HK) for c in range(NCORES)],
        axis=0)
    return out.reshape(B, S, H, DIM), res


def kernel(embeddings, hidden_states, Wv, bv, Wk, bk, g1, g2):
    out, _ = kernel_with_results(
        embeddings, hidden_states, Wv, bv, Wk, bk, g1, g2)
    return out
